# revision 18
# baseline (speedup 1.0000x reference)
"""Trainium2 Bass kernel for a dense pre-norm transformer block with ALiBi attention.

Reference semantics (B=2, T=2048, C=1024, H=16, HS=64):
    h  = LN1(x);  q,k,v = per-head projections of h
    wei = softmax(causal(q k^T / sqrt(HS) + alibi))
    x  = x + (concat_heads(wei @ v) @ Wproj + bproj)
    x  = x + (relu(LN2(x) @ W1 + b1) @ W2 + b2)

Distribution over 8 NeuronCores: 2-way data parallel over batch (quads
{0..3} and {4..7}) x 4-way tensor parallel over heads within each quad
(4 heads per core).  After attention each core holds its 4 heads' outputs
for all tokens; a small bf16 AllToAll within the quad transposes this to
"all 16 heads for my 512 tokens", after which the attention out-projection
and the FFN run fully local per core (no reduction collective needed).

On-device layout is feature-major ([feature, token]) throughout.  The host
pre-transposes inputs / post-transposes outputs, folds the LN gains/biases
into the adjacent weight matrices, and pre-scales Wk by 1/sqrt(HS).
ALiBi+causal masking is a multiplicative factor table
F[s,t] = exp(-slope*|t-s|) * (s<=t), precomputed on host per head.
The softmax denominator is fused into the AV matmul by appending a ones
column to each head's V block (65-wide stationary operand).
"""

import math

import numpy as np
import ml_dtypes

import concourse.bass as bass
import concourse.mybir as mybir
from concourse import bacc
from concourse.tile import TileContext
from concourse.bass_utils import run_bass_kernel_spmd

B, T, C, H, HS = 2, 2048, 1024, 16, 64
EPS = 1e-5
NCORES = 8
HPC = 4            # heads per core
TOK = 512          # tokens owned per core (FFN/output shard)
FW = 2432          # factor-table width: 384 + 1536 + 512
BF = mybir.dt.bfloat16
F32 = mybir.dt.float32
AF = mybir.ActivationFunctionType
ALU = mybir.AluOpType
NP_BF16 = ml_dtypes.bfloat16


def _alibi_slopes(n_head):
    n = 2 ** int(math.floor(math.log2(n_head)))
    m = np.power(2.0 ** (-8.0 / n), np.arange(1, n + 1))
    if n < n_head:
        m_hat = np.power(2.0 ** (-4.0 / n), np.arange(1, 1 + 2 * (n_head - n), 2))
        m = np.concatenate([m, m_hat])
    return m.astype(np.float64)


def _factor_table(slope):
    """F[i, u]: for tile (s0, t0), F[i, 384+(t0-s0)+j] = alibi*mask at s=s0+i, t=t0+j."""
    i = np.arange(128)[:, None]
    d = np.arange(FW)[None, :] - 384          # d = (t0-s0)+j;  t-s = d-i
    rel = d - i
    f = np.exp(-slope * np.abs(rel))
    f[rel < 0] = 0.0
    return f.astype(NP_BF16)


def build_bass():
    nc = bacc.Bacc("TRN2", debug=False, num_devices=NCORES)

    # ---- I/O ----
    xfm = nc.dram_tensor("xfm", [128, 8, T], F32, kind="ExternalInput")
    xown = nc.dram_tensor("xown", [128, 8, TOK], F32, kind="ExternalInput")
    wq = nc.dram_tensor("wq", [128, 8, 256], BF, kind="ExternalInput")
    wk = nc.dram_tensor("wk", [128, 8, 256], BF, kind="ExternalInput")
    wv = nc.dram_tensor("wv", [128, 8, 256], BF, kind="ExternalInput")
    bq = nc.dram_tensor("bq", [128, 2], F32, kind="ExternalInput")
    bk = nc.dram_tensor("bk", [128, 2], F32, kind="ExternalInput")
    bv = nc.dram_tensor("bv", [1, 256], F32, kind="ExternalInput")
    wp = nc.dram_tensor("wp", [128, 8, 1024], BF, kind="ExternalInput")
    bp = nc.dram_tensor("bp", [128, 8], F32, kind="ExternalInput")
    ft = nc.dram_tensor("ft", [HPC, 128, FW], BF, kind="ExternalInput")
    w1 = nc.dram_tensor("w1", [32, 128, 8, 128], BF, kind="ExternalInput")
    b1 = nc.dram_tensor("b1", [128, 32], F32, kind="ExternalInput")
    w2 = nc.dram_tensor("w2", [8, 128, 32, 128], BF, kind="ExternalInput")
    b2 = nc.dram_tensor("b2", [128, 8], F32, kind="ExternalInput")
    msk = nc.dram_tensor("msk", [128, 2], F32, kind="ExternalInput")
    y = nc.dram_tensor("y", [128, 8, TOK], F32, kind="ExternalOutput")

    with TileContext(nc) as tc:
        with (
            tc.tile_pool(name="const", bufs=1) as cp,
            tc.tile_pool(name="dram", bufs=1, space="DRAM") as dp,
        ):
            ones_bf = cp.tile([128, 1], BF)
            nc.vector.memset(ones_bf[:], 1.0)
            eps_t = cp.tile([1, 1], F32)
            nc.vector.memset(eps_t[:], EPS)
            bq_t = cp.tile([128, 2], F32, tag="bq")
            nc.sync.dma_start(bq_t[:], bq[:])
            bk_t = cp.tile([128, 2], F32, tag="bk")
            nc.sync.dma_start(bk_t[:], bk[:])
            bv_row = cp.tile([1, 256], F32, tag="bvr")
            nc.sync.dma_start(bv_row[:], bv[:])
            bv_b = cp.tile([128, 256], F32, tag="bvb")
            nc.gpsimd.partition_broadcast(bv_b[:], bv_row[:])
            bp_t = cp.tile([128, 8], F32, tag="bp")
            nc.sync.dma_start(bp_t[:], bp[:])
            b1_t = cp.tile([128, 32], F32, tag="b1")
            nc.sync.dma_start(b1_t[:], b1[:])
            b2_t = cp.tile([128, 8], F32, tag="b2")
            nc.sync.dma_start(b2_t[:], b2[:])
            xo_t = cp.tile([128, 8, TOK], F32, tag="xo")
            nc.sync.dma_start(xo_t[:], xown[:])
            wp_t = cp.tile([128, 8, 1024], BF, tag="wp")
            nc.sync.dma_start(wp_t[:], wp[:])

            msk_t = cp.tile([128, 2], F32, tag="msk")
            nc.sync.dma_start(msk_t[:], msk[:])

            # The intra-quad head->token transpose runs as an 8-way AllToAll
            # (4-core AllToAll is unsupported).  Each core stages its block
            # masked by its quad indicator into BOTH the low (chunks 0-3,
            # quad-0 destinations) and high (chunks 4-7, quad-1) slots; the
            # wrong-quad copy is zeros, so receivers just add the halves.
            a2a_in = dp.tile([8, 256, TOK], BF)
            a2a_out = dp.tile([8, 256, TOK], BF)

            # -------- per-512-chunk LayerNorm stats -> h = (x-mu)*rstd --------
            # Feature-major: mean/var over the partition(feature) dim via
            # ones-matmuls; rstd via Sqrt + vector reciprocal (no Ln/Exp
            # activation-table thrash); normalize in bf16 split across
            # GpSimd (sub) and Vector (mult).
            def layernorm_fm(xb_sb, W, lp, lps, rowp, write_out):
                xsq = lp.tile([128, 8, W], BF, tag="ln_xsq", bufs=2)
                nc.gpsimd.tensor_tensor(xsq[:], xb_sb[:], xb_sb[:], ALU.mult)
                sx = lps.tile([1, W], F32, tag="ln_sx", bufs=1)
                sq = lps.tile([1, W], F32, tag="ln_sq", bufs=1)
                for kc in range(8):
                    nc.tensor.matmul(sx[:], ones_bf[:], xb_sb[:, kc, :],
                                     start=(kc == 0), stop=(kc == 7))
                for kc in range(8):
                    nc.tensor.matmul(sq[:], ones_bf[:], xsq[:, kc, :],
                                     start=(kc == 0), stop=(kc == 7))
                mu = rowp.tile([1, W], F32, tag="ln_mu", bufs=2)
                nc.scalar.mul(mu[:], sx[:], 1.0 / C)
                musq = rowp.tile([1, W], F32, tag="ln_musq", bufs=2)
                nc.vector.tensor_tensor(musq[:], mu[:], mu[:], ALU.mult)
                sd = rowp.tile([1, W], F32, tag="ln_sd", bufs=2)
                nc.vector.scalar_tensor_tensor(sd[:], sq[:], 1.0 / C, musq[:],
                                               ALU.mult, ALU.subtract)
                nc.scalar.activation(sd[:], sd[:], AF.Sqrt, bias=eps_t[:])
                rstd = rowp.tile([1, W], F32, tag="ln_rstd", bufs=2)
                nc.vector.reciprocal(rstd[:], sd[:])
                mu_bf = rowp.tile([1, W], BF, tag="ln_mubf", bufs=2)
                nc.vector.tensor_copy(mu_bf[:], mu[:])
                mub = lp.tile([128, W], BF, tag="ln_mub", bufs=2)
                nc.gpsimd.partition_broadcast(mub[:], mu_bf[:])
                rsb = lp.tile([128, W], F32, tag="ln_rsb", bufs=2)
                nc.gpsimd.partition_broadcast(rsb[:], rstd[:])
                for kc in range(8):
                    tmp = lp.tile([128, W], BF, tag="ln_tmp", bufs=4)
                    if kc % 2 == 0:
                        nc.gpsimd.tensor_sub(tmp[:], xb_sb[:, kc, :], mub[:])
                        nc.vector.tensor_tensor(write_out(kc), tmp[:], rsb[:],
                                                ALU.mult)
                    else:
                        nc.vector.tensor_sub(tmp[:], xb_sb[:, kc, :], mub[:])
                        nc.gpsimd.tensor_tensor(write_out(kc), tmp[:], rsb[:],
                                                ALU.mult)

            with tc.tile_pool(name="qkvpool", bufs=1) as qp:
                qfm = qp.tile([128, 2, T], BF, tag="qfm")
                kfm = qp.tile([128, 2, T], BF, tag="kfm")
                v_t = qp.tile([128, 16, HPC, 65], BF, tag="v")
                nc.vector.memset(v_t[:, :, :, 64:65], 1.0)

                # ------- LN1 + QKV, pipelined per 512-token chunk -------
                with (
                    tc.tile_pool(name="hpool", bufs=1) as hp,
                    tc.tile_pool(name="xin", bufs=2) as xp,
                    tc.tile_pool(name="lnp", bufs=1) as lp,
                    tc.tile_pool(name="lnrow", bufs=1) as rowp,
                    tc.tile_pool(name="lnps", bufs=2, space="PSUM") as lps,
                    tc.tile_pool(name="wqkv", bufs=1) as wqp,
                    tc.tile_pool(name="qkps", bufs=4, space="PSUM") as qps,
                ):
                    wq_t = wqp.tile([128, 8, 256], BF, tag="wq")
                    nc.sync.dma_start(wq_t[:], wq[:])
                    wk_t = wqp.tile([128, 8, 256], BF, tag="wk")
                    nc.sync.dma_start(wk_t[:], wk[:])
                    wv_t = wqp.tile([128, 8, 256], BF, tag="wv")
                    nc.sync.dma_start(wv_t[:], wv[:])
                    h_t = hp.tile([128, 8, T], BF, tag="h")

                    for ch in range(4):
                        tsl = slice(ch * 512, (ch + 1) * 512)
                        xc = xp.tile([128, 8, 512], F32, tag="xc")
                        nc.sync.dma_start(xc[:], xfm[:, :, tsl])
                        xb = xp.tile([128, 8, 512], BF, tag="xb")
                        nc.vector.tensor_copy(xb[:], xc[:])
                        layernorm_fm(
                            xb, 512, lp, lps, rowp,
                            lambda kc, ch=ch: h_t[:, kc, ch * 512:(ch + 1) * 512])

                        # Q,K for this chunk
                        for p in range(2):
                            ps = qps.tile([128, 512], F32, tag="qk_ps", bufs=3)
                            for kc in range(8):
                                nc.tensor.matmul(
                                    ps[:], wq_t[:, kc, p * 128:(p + 1) * 128],
                                    h_t[:, kc, tsl],
                                    start=(kc == 0), stop=(kc == 7))
                            nc.scalar.add(qfm[:, p, tsl], ps[:], bq_t[:, p:p + 1])
                            ps2 = qps.tile([128, 512], F32, tag="qk_ps",
                                           bufs=3)
                            for kc in range(8):
                                nc.tensor.matmul(
                                    ps2[:], wk_t[:, kc, p * 128:(p + 1) * 128],
                                    h_t[:, kc, tsl],
                                    start=(kc == 0), stop=(kc == 7))
                            nc.scalar.add(kfm[:, p, tsl], ps2[:], bk_t[:, p:p + 1])
                        # V for this chunk (token-major, 128-token blocks)
                        for t4 in range(4):
                            tch = ch * 4 + t4
                            psv = qps.tile([128, 256], F32, tag="v_ps", bufs=2)
                            for kc in range(8):
                                nc.tensor.matmul(
                                    psv[:], h_t[:, kc, tch * 128:(tch + 1) * 128],
                                    wv_t[:, kc, :],
                                    start=(kc == 0), stop=(kc == 7))
                            nc.vector.tensor_add(v_t[:, tch, :, 0:64], psv[:],
                                                 bv_b[:])

                # ---------- Attention ----------  (h freed; qkv + F live)
                with (
                    tc.tile_pool(name="fpool", bufs=1) as fp,
                    tc.tile_pool(name="scps", bufs=2, space="PSUM") as scp,
                    tc.tile_pool(name="oaps", bufs=2, space="PSUM") as oap,
                    tc.tile_pool(name="attp", bufs=3) as atp,
                    tc.tile_pool(name="onrm", bufs=1) as onp,
                ):
                    f_t = []
                    for hh in range(HPC):
                        f = fp.tile([128, FW], BF, tag=f"ft{hh}")
                        nc.sync.dma_start(f[:], ft[hh])
                        f_t.append(f)

                    for p in range(2):
                        for tcn in range(4):
                            t0 = tcn * 512
                            tsl = slice(t0, t0 + 512)
                            nums = []
                            for hh in range(2):
                                numt = oap.tile([65, 512], F32, tag=f"num{hh}")
                                nums.append(numt)
                            ns = 4 * (tcn + 1)
                            for si in range(ns):
                                s0 = si * 128
                                sc = scp.tile([128, 2, 512], F32, tag="sc")
                                at = atp.tile([128, 2, 512], BF, tag="at")
                                for hh in range(2):
                                    pb = 64 * hh
                                    nc.tensor.matmul(
                                        sc[:, hh, :],
                                        kfm[pb:pb + 64, p, s0:s0 + 128],
                                        qfm[pb:pb + 64, p, tsl],
                                        start=True, stop=True)
                                nc.scalar.activation(at[:], sc[:], AF.Exp)
                                dlt = t0 - s0 + 384
                                nc.vector.tensor_tensor(
                                    at[:, 0, :], at[:, 0, :],
                                    f_t[2 * p][:, dlt:dlt + 512], ALU.mult)
                                nc.gpsimd.tensor_tensor(
                                    at[:, 1, :], at[:, 1, :],
                                    f_t[2 * p + 1][:, dlt:dlt + 512], ALU.mult)
                                st, sp_ = (si == 0), (si == ns - 1)
                                for hh in range(2):
                                    nc.tensor.matmul(
                                        nums[hh][:],
                                        v_t[:, si, 2 * p + hh, :],
                                        at[:, hh, :],
                                        start=st, stop=sp_)
                            # normalize (num/den) and stage for AllToAll
                            for hh in range(2):
                                dsb = onp.tile([65, 512], F32, tag="dsb", bufs=3)
                                nc.vector.tensor_copy(dsb[64:65, :],
                                                      nums[hh][64:65, :])
                                den = onp.tile([1, 512], F32, tag="den", bufs=3)
                                nc.sync.dma_start(den[:], dsb[64:65, :])
                                rec = onp.tile([1, 512], F32, tag="rec", bufs=3)
                                nc.vector.reciprocal(rec[:], den[:])
                                rb = onp.tile([64, 512], F32, tag="rb", bufs=3)
                                nc.gpsimd.partition_broadcast(rb[:], rec[:])
                                ofh = onp.tile([64, 512], BF, tag="ofh", bufs=3)
                                nc.vector.tensor_tensor(
                                    ofh[:], nums[hh][0:64, :], rb[:], ALU.mult)
                                rows = slice(p * 128 + 64 * hh,
                                             p * 128 + 64 * hh + 64)
                                olo = onp.tile([64, 512], BF, tag="olo", bufs=3)
                                nc.vector.tensor_scalar_mul(
                                    olo[:], ofh[:], msk_t[0:64, 0:1])
                                nc.sync.dma_start(a2a_in[tcn, rows, :], olo[:])
                                ohi = onp.tile([64, 512], BF, tag="ohi", bufs=3)
                                nc.gpsimd.tensor_scalar_mul(
                                    ohi[:], ofh[:], msk_t[0:64, 1:2])
                                nc.sync.dma_start(a2a_in[4 + tcn, rows, :],
                                                  ohi[:])

            # ---- tiny bf16 AllToAll within each quad: heads -> tokens ----
            nc.gpsimd.collective_compute(
                "AllToAll", ALU.bypass,
                replica_groups=[[0, 1, 2, 3, 4, 5, 6, 7]],
                ins=[a2a_in.opt()], outs=[a2a_out.opt()])

            # ------- out-projection + residual + LN2 + FFN on own tokens -------
            with tc.tile_pool(name="x2pool", bufs=1) as x2p:
                x2own = x2p.tile([128, 8, TOK], F32, tag="x2own")
                x2b = x2p.tile([128, 8, TOK], BF, tag="x2b")

                with (
                    tc.tile_pool(name="ofl", bufs=1) as ofp,
                    tc.tile_pool(name="oflin", bufs=4) as ofi,
                    tc.tile_pool(name="prps", bufs=3, space="PSUM") as prp,
                ):
                    ofull = ofp.tile([128, 8, TOK], BF, tag="ofull")
                    for j in range(4):
                        for pp in range(2):
                            rows = slice(128 * pp, 128 * (pp + 1))
                            olo = ofi.tile([128, TOK], BF, tag="glo")
                            nc.sync.dma_start(olo[:], a2a_out[j, rows, :])
                            ohi = ofi.tile([128, TOK], BF, tag="ghi")
                            nc.sync.dma_start(ohi[:], a2a_out[4 + j, rows, :])
                            nc.vector.tensor_add(
                                ofull[:, 2 * j + pp, :], olo[:], ohi[:])
                    for m in range(8):
                        ps = prp.tile([128, TOK], F32, tag="pr_ps")
                        for kc in range(8):
                            nc.tensor.matmul(
                                ps[:], wp_t[:, kc, m * 128:(m + 1) * 128],
                                ofull[:, kc, :],
                                start=(kc == 0), stop=(kc == 7))
                        nc.vector.scalar_tensor_tensor(
                            x2own[:, m, :], ps[:], bp_t[:, m:m + 1],
                            xo_t[:, m, :], ALU.add, ALU.add)
                        nc.gpsimd.tensor_copy(x2b[:, m, :], x2own[:, m, :])

                with tc.tile_pool(name="ffn", bufs=1) as ffp:
                    h2 = ffp.tile([128, 8, TOK], BF, tag="h2")
                    with (
                        tc.tile_pool(name="l2p", bufs=1) as l2p,
                        tc.tile_pool(name="l2row", bufs=1) as l2row,
                        tc.tile_pool(name="l2ps", bufs=2, space="PSUM") as l2ps,
                    ):
                        layernorm_fm(x2b, TOK, l2p, l2ps, l2row,
                                     lambda kc: h2[:, kc, :])

                    mid = ffp.tile([128, 32, TOK], BF, tag="mid")
                    with (
                        tc.tile_pool(name="w1p", bufs=4) as w1p,
                        tc.tile_pool(name="ffps", bufs=4, space="PSUM") as fps,
                    ):
                        for m in range(32):
                            w1t = w1p.tile([128, 8, 128], BF, tag="w1t")
                            nc.sync.dma_start(w1t[:], w1[m])
                            ps = fps.tile([128, TOK], F32, tag="ff_ps")
                            for kc in range(8):
                                nc.tensor.matmul(
                                    ps[:], w1t[:, kc, :], h2[:, kc, :],
                                    start=(kc == 0), stop=(kc == 7))
                            nc.scalar.activation(mid[:, m, :], ps[:], AF.Relu,
                                                 bias=b1_t[:, m:m + 1])
                    with (
                        tc.tile_pool(name="w2p", bufs=3) as w2p,
                        tc.tile_pool(name="ff2ps", bufs=4, space="PSUM") as fp2,
                        tc.tile_pool(name="yst", bufs=3) as ysp,
                    ):
                        for m in range(8):
                            w2t = w2p.tile([128, 32, 128], BF, tag="w2t")
                            nc.sync.dma_start(w2t[:], w2[m])
                            ps = fp2.tile([128, TOK], F32, tag="ff2_ps")
                            for kc in range(32):
                                nc.tensor.matmul(
                                    ps[:], w2t[:, kc, :], mid[:, kc, :],
                                    start=(kc == 0), stop=(kc == 31))
                            ym = ysp.tile([128, TOK], F32, tag="ym")
                            nc.vector.scalar_tensor_tensor(
                                ym[:], ps[:], b2_t[:, m:m + 1],
                                x2own[:, m, :], ALU.add, ALU.add)
                            nc.sync.dma_start(y[:, m, :], ym[:])

    nc.compile()
    return nc


_NC_CACHE = None


def _get_nc():
    global _NC_CACHE
    if _NC_CACHE is None:
        _NC_CACHE = build_bass()
    return _NC_CACHE


def _fm_tile(a):
    """[C, N] -> [128, C//128, N] (partition-major feature tiling)."""
    Cd, N = a.shape
    return np.ascontiguousarray(a.reshape(Cd // 128, 128, N).transpose(1, 0, 2))


def prepare_inputs(x, Wq, Wk, Wv, Wproj, bproj, ln1_g, ln1_b, ln2_g, ln2_b,
                   W1, b1, W2, b2):
    """Build the 8 per-core input dicts (all numpy, host side)."""
    x = np.asarray(x, np.float32)
    f32 = lambda a: np.asarray(a, np.float32)
    Wq, Wk, Wv = f32(Wq), f32(Wk), f32(Wv)
    Wproj, bproj = f32(Wproj), f32(bproj)
    ln1_g, ln1_b, ln2_g, ln2_b = f32(ln1_g), f32(ln1_b), f32(ln2_g), f32(ln2_b)
    W1, b1, W2, b2 = f32(W1), f32(b1), f32(W2), f32(b2)

    slopes = _alibi_slopes(H)

    # fold LN1 gain/bias into the QKV weights:  h = ln_raw*g + b
    WqF = Wq * ln1_g[None, :, None]      # [H, C, HS]
    WkF = Wk * ln1_g[None, :, None] * (HS ** -0.5)   # fold 1/sqrt(HS) into K
    WvF = Wv * ln1_g[None, :, None]
    bqF = np.einsum("c,hcd->hd", ln1_b, WqF)   # [H, HS]
    bkF = np.einsum("c,hcd->hd", ln1_b, WkF)
    bvF = np.einsum("c,hcd->hd", ln1_b, WvF)
    # fold LN2 gain/bias into W1
    W1F = W1 * ln2_g[:, None]
    b1F = b1 + ln2_b @ W1F

    w1h = np.ascontiguousarray(
        W1F.astype(NP_BF16).reshape(8, 128, 32, 128).transpose(2, 1, 0, 3))
    w2h = np.ascontiguousarray(
        W2.astype(NP_BF16).reshape(32, 128, 8, 128).transpose(2, 1, 0, 3))
    b1h = np.ascontiguousarray(b1F.reshape(32, 128).T)
    b2h = np.ascontiguousarray(b2.reshape(8, 128).T)
    bph = np.ascontiguousarray(bproj.reshape(8, 128).T)
    wph = _fm_tile(Wproj.astype(NP_BF16))      # full [128, 8, 1024]

    in_maps = []
    for c in range(NCORES):
        b = c // 4
        g = c % 4
        mskh = np.zeros((128, 2), np.float32)
        mskh[:, b] = 1.0
        heads = range(4 * g, 4 * g + 4)
        xb = x[b].T                                    # [C, T] feature-major
        wq_own = np.concatenate([WqF[h] for h in heads], axis=1)   # [C, 256]
        wk_own = np.concatenate([WkF[h] for h in heads], axis=1)
        wv_own = np.concatenate([WvF[h] for h in heads], axis=1)
        bq_own = np.concatenate([bqF[h] for h in heads])           # [256]
        bk_own = np.concatenate([bkF[h] for h in heads])
        bv_own = np.concatenate([bvF[h] for h in heads])
        fts = np.stack([_factor_table(slopes[h]) for h in heads])  # [4,128,FW]

        in_maps.append({
            "xfm": _fm_tile(xb),
            "xown": _fm_tile(xb[:, g * TOK:(g + 1) * TOK]),
            "wq": _fm_tile(wq_own.astype(NP_BF16)),
            "wk": _fm_tile(wk_own.astype(NP_BF16)),
            "wv": _fm_tile(wv_own.astype(NP_BF16)),
            "bq": np.ascontiguousarray(bq_own.reshape(2, 128).T.astype(np.float32)),
            "bk": np.ascontiguousarray(bk_own.reshape(2, 128).T.astype(np.float32)),
            "bv": bv_own[None, :].astype(np.float32),
            "wp": wph,
            "bp": bph,
            "ft": fts,
            "w1": w1h,
            "b1": b1h,
            "w2": w2h,
            "b2": b2h,
            "msk": mskh,
        })
    return in_maps


def assemble_output(results):
    out = np.empty((B, T, C), np.float32)
    for c in range(NCORES):
        b, g = c // 4, c % 4
        yc = results[c]["y"]                        # [128, 8, TOK]
        yc = yc.transpose(1, 0, 2).reshape(C, TOK)  # [C, TOK]
        out[b, g * TOK:(g + 1) * TOK, :] = yc.T
    return out


def kernel(**inputs):
    nc = _get_nc()
    in_maps = prepare_inputs(**inputs)
    res = run_bass_kernel_spmd(nc, in_maps, core_ids=list(range(NCORES)))
    return assemble_output(res.results)


if __name__ == "__main__":
    import reference
    ins = {k: np.asarray(v) for k, v in reference.setup_inputs().items()}
    exp = np.asarray(reference.reference(**ins))
    got = kernel(**ins)
    err = np.linalg.norm(got - exp) / np.linalg.norm(exp)
    print("Relative error:", err)


# revision 23
# speedup vs baseline: 1.0841x; 1.0841x over previous
"""Trainium2 Bass kernel for a dense pre-norm transformer block with ALiBi attention.

Reference semantics (B=2, T=2048, C=1024, H=16, HS=64):
    h  = LN1(x);  q,k,v = per-head projections of h
    wei = softmax(causal(q k^T / sqrt(HS) + alibi))
    x  = x + (concat_heads(wei @ v) @ Wproj + bproj)
    x  = x + (relu(LN2(x) @ W1 + b1) @ W2 + b2)

Distribution over 8 NeuronCores: 2-way data parallel over batch (quads
{0..3} and {4..7}) x 4-way tensor parallel over heads within each quad
(4 heads per core).  After attention each core holds its 4 heads' outputs
for all tokens; a small bf16 AllToAll within the quad transposes this to
"all 16 heads for my 512 tokens", after which the attention out-projection
and the FFN run fully local per core (no reduction collective needed).

On-device layout is feature-major ([feature, token]) throughout.  The host
pre-transposes inputs / post-transposes outputs, folds the LN gains/biases
into the adjacent weight matrices, and pre-scales Wk by 1/sqrt(HS).
ALiBi+causal masking is a multiplicative factor table
F[s,t] = exp(-slope*|t-s|) * (s<=t), precomputed on host per head.
The softmax denominator is fused into the AV matmul by appending a ones
column to each head's V block (65-wide stationary operand).
"""

import math

import numpy as np
import ml_dtypes

import concourse.bass as bass
import concourse.mybir as mybir
from concourse import bacc
from concourse.tile import TileContext
from concourse.bass_utils import run_bass_kernel_spmd

B, T, C, H, HS = 2, 2048, 1024, 16, 64
EPS = 1e-5
NCORES = 8
HPC = 4            # heads per core
TOK = 512          # tokens owned per core (FFN/output shard)
FW = 2432          # factor-table width: 384 + 1536 + 512
BF = mybir.dt.bfloat16
F32 = mybir.dt.float32
AF = mybir.ActivationFunctionType
ALU = mybir.AluOpType
NP_BF16 = ml_dtypes.bfloat16


def _alibi_slopes(n_head):
    n = 2 ** int(math.floor(math.log2(n_head)))
    m = np.power(2.0 ** (-8.0 / n), np.arange(1, n + 1))
    if n < n_head:
        m_hat = np.power(2.0 ** (-4.0 / n), np.arange(1, 1 + 2 * (n_head - n), 2))
        m = np.concatenate([m, m_hat])
    return m.astype(np.float64)


def _factor_table(slope):
    """F[i, u]: for tile (s0, t0), F[i, 384+(t0-s0)+j] = alibi*mask at s=s0+i, t=t0+j."""
    i = np.arange(128)[:, None]
    d = np.arange(FW)[None, :] - 384          # d = (t0-s0)+j;  t-s = d-i
    rel = d - i
    f = np.exp(-slope * np.abs(rel))
    f[rel < 0] = 0.0
    return f.astype(NP_BF16)


def build_bass():
    nc = bacc.Bacc("TRN2", debug=False, num_devices=NCORES)

    # ---- I/O ----
    xfm = nc.dram_tensor("xfm", [128, 8, T], F32, kind="ExternalInput")
    xown = nc.dram_tensor("xown", [128, 8, TOK], F32, kind="ExternalInput")
    wq = nc.dram_tensor("wq", [128, 8, 256], BF, kind="ExternalInput")
    wk = nc.dram_tensor("wk", [128, 8, 256], BF, kind="ExternalInput")
    wv = nc.dram_tensor("wv", [128, 8, 256], BF, kind="ExternalInput")
    bq = nc.dram_tensor("bq", [128, 2], F32, kind="ExternalInput")
    bk = nc.dram_tensor("bk", [128, 2], F32, kind="ExternalInput")
    bv = nc.dram_tensor("bv", [1, 256], F32, kind="ExternalInput")
    wp = nc.dram_tensor("wp", [128, 8, 1024], BF, kind="ExternalInput")
    bp = nc.dram_tensor("bp", [128, 8], F32, kind="ExternalInput")
    ft = nc.dram_tensor("ft", [HPC, 128, FW], BF, kind="ExternalInput")
    w1 = nc.dram_tensor("w1", [32, 128, 8, 128], BF, kind="ExternalInput")
    b1 = nc.dram_tensor("b1", [128, 32], F32, kind="ExternalInput")
    w2 = nc.dram_tensor("w2", [8, 128, 32, 128], BF, kind="ExternalInput")
    b2 = nc.dram_tensor("b2", [128, 8], F32, kind="ExternalInput")
    msk = nc.dram_tensor("msk", [128, 2], F32, kind="ExternalInput")
    y = nc.dram_tensor("y", [128, 8, TOK], F32, kind="ExternalOutput")

    with TileContext(nc) as tc:
        with (
            tc.tile_pool(name="const", bufs=1) as cp,
            tc.tile_pool(name="dram", bufs=1, space="DRAM") as dp,
        ):
            ones_bf = cp.tile([128, 1], BF)
            nc.vector.memset(ones_bf[:], 1.0)
            eps_t = cp.tile([1, 1], F32)
            nc.vector.memset(eps_t[:], EPS)
            bq_t = cp.tile([128, 2], F32, tag="bq")
            nc.sync.dma_start(bq_t[:], bq[:])
            bk_t = cp.tile([128, 2], F32, tag="bk")
            nc.sync.dma_start(bk_t[:], bk[:])
            bv_row = cp.tile([1, 256], F32, tag="bvr")
            nc.sync.dma_start(bv_row[:], bv[:])
            bv_b = cp.tile([128, 256], F32, tag="bvb")
            nc.gpsimd.partition_broadcast(bv_b[:], bv_row[:])
            bp_t = cp.tile([128, 8], F32, tag="bp")
            nc.sync.dma_start(bp_t[:], bp[:])
            b1_t = cp.tile([128, 32], F32, tag="b1")
            nc.sync.dma_start(b1_t[:], b1[:])
            b2_t = cp.tile([128, 8], F32, tag="b2")
            nc.sync.dma_start(b2_t[:], b2[:])
            # (loaded later, after the x chunks, to keep the DMA queue clear
            # for the LN1 input at kernel start)
            xo_t = cp.tile([128, 8, TOK], F32, tag="xo")
            wp_t = cp.tile([128, 8, 1024], BF, tag="wp")

            msk_t = cp.tile([128, 2], F32, tag="msk")
            nc.sync.dma_start(msk_t[:], msk[:])

            # The intra-quad head->token transpose runs as an 8-way AllToAll
            # (4-core AllToAll is unsupported).  Each core stages its block
            # masked by its quad indicator into BOTH the low (chunks 0-3,
            # quad-0 destinations) and high (chunks 4-7, quad-1) slots; the
            # wrong-quad copy is zeros, so receivers just add the halves.
            a2a_in = dp.tile([8, 256, TOK], BF)
            a2a_out = dp.tile([8, 256, TOK], BF)

            # -------- per-512-chunk LayerNorm stats -> h = (x-mu)*rstd --------
            # Feature-major: mean/var over the partition(feature) dim via
            # ones-matmuls; rstd via Sqrt + vector reciprocal (no Ln/Exp
            # activation-table thrash); normalize in bf16 split across
            # GpSimd (sub) and Vector (mult).
            def layernorm_fm(xb_sb, W, lp, lps, rowp, write_out):
                xsq = lp.tile([128, 8, W], BF, tag="ln_xsq", bufs=2)
                nc.gpsimd.tensor_tensor(xsq[:], xb_sb[:], xb_sb[:], ALU.mult)
                sx = lps.tile([1, W], F32, tag="ln_sx", bufs=1)
                sq = lps.tile([1, W], F32, tag="ln_sq", bufs=1)
                for kc in range(8):
                    nc.tensor.matmul(sx[:], ones_bf[:], xb_sb[:, kc, :],
                                     start=(kc == 0), stop=(kc == 7))
                for kc in range(8):
                    nc.tensor.matmul(sq[:], ones_bf[:], xsq[:, kc, :],
                                     start=(kc == 0), stop=(kc == 7))
                mu = rowp.tile([1, W], F32, tag="ln_mu", bufs=2)
                nc.scalar.mul(mu[:], sx[:], 1.0 / C)
                musq = rowp.tile([1, W], F32, tag="ln_musq", bufs=2)
                nc.vector.tensor_tensor(musq[:], mu[:], mu[:], ALU.mult)
                sd = rowp.tile([1, W], F32, tag="ln_sd", bufs=2)
                nc.vector.scalar_tensor_tensor(sd[:], sq[:], 1.0 / C, musq[:],
                                               ALU.mult, ALU.subtract)
                nc.scalar.activation(sd[:], sd[:], AF.Sqrt, bias=eps_t[:])
                rstd = rowp.tile([1, W], F32, tag="ln_rstd", bufs=2)
                nc.vector.reciprocal_approx_fast(rstd[:], sd[:])
                mu_bf = rowp.tile([1, W], BF, tag="ln_mubf", bufs=2)
                nc.vector.tensor_copy(mu_bf[:], mu[:])
                rstd_bf = rowp.tile([1, W], BF, tag="ln_rstdbf", bufs=2)
                nc.vector.tensor_copy(rstd_bf[:], rstd[:])
                mub = lp.tile([128, W], BF, tag="ln_mub", bufs=2)
                nc.gpsimd.partition_broadcast(mub[:], mu_bf[:])
                rsb = lp.tile([128, W], BF, tag="ln_rsb", bufs=2)
                nc.gpsimd.partition_broadcast(rsb[:], rstd_bf[:])
                for kc in range(8):
                    tmp = lp.tile([128, W], BF, tag="ln_tmp", bufs=4)
                    if kc % 2 == 0:
                        nc.gpsimd.tensor_sub(tmp[:], xb_sb[:, kc, :], mub[:])
                    else:
                        nc.vector.tensor_sub(tmp[:], xb_sb[:, kc, :], mub[:])
                    nc.vector.tensor_tensor(write_out(kc), tmp[:], rsb[:],
                                            ALU.mult)

            with tc.tile_pool(name="qkvpool", bufs=1) as qp:
                qfm = qp.tile([128, 2, T], BF, tag="qfm")
                kfm = qp.tile([128, 2, T], BF, tag="kfm")
                v_t = qp.tile([128, 16, HPC, 65], BF, tag="v")
                nc.vector.memset(v_t[:, :, :, 64:65], 1.0)

                # ------- LN1 + QKV, pipelined per 512-token chunk -------
                with (
                    tc.tile_pool(name="hpool", bufs=1) as hp,
                    tc.tile_pool(name="xin", bufs=2) as xp,
                    tc.tile_pool(name="lnp", bufs=1) as lp,
                    tc.tile_pool(name="lnrow", bufs=1) as rowp,
                    tc.tile_pool(name="lnps", bufs=2, space="PSUM") as lps,
                    tc.tile_pool(name="wqkv", bufs=1) as wqp,
                    tc.tile_pool(name="qkps", bufs=4, space="PSUM") as qps,
                ):
                    wq_t = wqp.tile([128, 8, 256], BF, tag="wq")
                    nc.sync.dma_start(wq_t[:], wq[:])
                    wk_t = wqp.tile([128, 8, 256], BF, tag="wk")
                    nc.sync.dma_start(wk_t[:], wk[:])
                    wv_t = wqp.tile([128, 8, 256], BF, tag="wv")
                    nc.sync.dma_start(wv_t[:], wv[:])
                    h_t = hp.tile([128, 8, T], BF, tag="h")

                    for ch in range(4):
                        tsl = slice(ch * 512, (ch + 1) * 512)
                        xc = xp.tile([128, 8, 512], F32, tag="xc")
                        nc.sync.dma_start(xc[:], xfm[:, :, tsl])
                        xb = xp.tile([128, 8, 512], BF, tag="xb")
                        nc.vector.tensor_copy(xb[:], xc[:])
                        layernorm_fm(
                            xb, 512, lp, lps, rowp,
                            lambda kc, ch=ch: h_t[:, kc, ch * 512:(ch + 1) * 512])

                        # Q,K for this chunk
                        for p in range(2):
                            ps = qps.tile([128, 512], F32, tag="qk_ps", bufs=3)
                            for kc in range(8):
                                nc.tensor.matmul(
                                    ps[:], wq_t[:, kc, p * 128:(p + 1) * 128],
                                    h_t[:, kc, tsl],
                                    start=(kc == 0), stop=(kc == 7))
                            nc.scalar.add(qfm[:, p, tsl], ps[:], bq_t[:, p:p + 1])
                            ps2 = qps.tile([128, 512], F32, tag="qk_ps",
                                           bufs=3)
                            for kc in range(8):
                                nc.tensor.matmul(
                                    ps2[:], wk_t[:, kc, p * 128:(p + 1) * 128],
                                    h_t[:, kc, tsl],
                                    start=(kc == 0), stop=(kc == 7))
                            nc.scalar.add(kfm[:, p, tsl], ps2[:], bk_t[:, p:p + 1])
                        # V for this chunk (token-major, 128-token blocks)
                        for t4 in range(4):
                            tch = ch * 4 + t4
                            psv = qps.tile([128, 256], F32, tag="v_ps", bufs=2)
                            for kc in range(8):
                                nc.tensor.matmul(
                                    psv[:], h_t[:, kc, tch * 128:(tch + 1) * 128],
                                    wv_t[:, kc, :],
                                    start=(kc == 0), stop=(kc == 7))
                            nc.vector.tensor_add(v_t[:, tch, :, 0:64], psv[:],
                                                 bv_b[:])

                # ---------- Attention ----------  (h freed; qkv + F live)
                with (
                    tc.tile_pool(name="fpool", bufs=1) as fp,
                    tc.tile_pool(name="scps", bufs=2, space="PSUM") as scp,
                    tc.tile_pool(name="oaps", bufs=2, space="PSUM") as oap,
                    tc.tile_pool(name="attp", bufs=3) as atp,
                    tc.tile_pool(name="onrm", bufs=1) as onp,
                ):
                    f_t = []
                    for hh in range(HPC):
                        f = fp.tile([128, FW], BF, tag=f"ft{hh}")
                        nc.sync.dma_start(f[:], ft[hh])
                        f_t.append(f)
                    nc.sync.dma_start(xo_t[:], xown[:])
                    nc.sync.dma_start(wp_t[:], wp[:])

                    for p in range(2):
                        for tcn in range(4):
                            t0 = tcn * 512
                            tsl = slice(t0, t0 + 512)
                            nums = []
                            for hh in range(2):
                                numt = oap.tile([65, 512], F32, tag=f"num{hh}")
                                nums.append(numt)
                            ns = 4 * (tcn + 1)
                            for si in range(ns):
                                s0 = si * 128
                                sc = scp.tile([128, 2, 512], F32, tag="sc")
                                at = atp.tile([128, 2, 512], BF, tag="at")
                                for hh in range(2):
                                    pb = 64 * hh
                                    nc.tensor.matmul(
                                        sc[:, hh, :],
                                        kfm[pb:pb + 64, p, s0:s0 + 128],
                                        qfm[pb:pb + 64, p, tsl],
                                        start=True, stop=True)
                                nc.scalar.activation(at[:], sc[:], AF.Exp)
                                dlt = t0 - s0 + 384
                                am = atp.tile([128, 2, 512], BF, tag="am")
                                nc.vector.tensor_tensor(
                                    am[:, 0, :], at[:, 0, :],
                                    f_t[2 * p][:, dlt:dlt + 512], ALU.mult)
                                nc.gpsimd.tensor_tensor(
                                    am[:, 1, :], at[:, 1, :],
                                    f_t[2 * p + 1][:, dlt:dlt + 512], ALU.mult)
                                st, sp_ = (si == 0), (si == ns - 1)
                                for hh in range(2):
                                    nc.tensor.matmul(
                                        nums[hh][:],
                                        v_t[:, si, 2 * p + hh, :],
                                        am[:, hh, :],
                                        start=st, stop=sp_)
                            # normalize (num/den) and stage for AllToAll
                            for hh in range(2):
                                dsb = onp.tile([65, 512], F32, tag="dsb", bufs=3)
                                nc.vector.tensor_copy(dsb[64:65, :],
                                                      nums[hh][64:65, :])
                                den = onp.tile([1, 512], F32, tag="den", bufs=3)
                                nc.sync.dma_start(den[:], dsb[64:65, :])
                                rec = onp.tile([1, 512], F32, tag="rec", bufs=3)
                                nc.vector.reciprocal_approx_fast(rec[:], den[:])
                                rb = onp.tile([64, 512], F32, tag="rb", bufs=3)
                                nc.gpsimd.partition_broadcast(rb[:], rec[:])
                                ofh = onp.tile([64, 512], BF, tag="ofh", bufs=3)
                                nc.vector.tensor_tensor(
                                    ofh[:], nums[hh][0:64, :], rb[:], ALU.mult)
                                rows = slice(p * 128 + 64 * hh,
                                             p * 128 + 64 * hh + 64)
                                olo = onp.tile([64, 512], BF, tag="olo", bufs=3)
                                nc.vector.tensor_scalar_mul(
                                    olo[:], ofh[:], msk_t[0:64, 0:1])
                                nc.sync.dma_start(a2a_in[tcn, rows, :], olo[:])
                                ohi = onp.tile([64, 512], BF, tag="ohi", bufs=3)
                                nc.gpsimd.tensor_scalar_mul(
                                    ohi[:], ofh[:], msk_t[0:64, 1:2])
                                nc.sync.dma_start(a2a_in[4 + tcn, rows, :],
                                                  ohi[:])

            # ---- tiny bf16 AllToAll within each quad: heads -> tokens ----
            nc.gpsimd.collective_compute(
                "AllToAll", ALU.bypass,
                replica_groups=[[0, 1, 2, 3, 4, 5, 6, 7]],
                ins=[a2a_in.opt()], outs=[a2a_out.opt()])

            # ------- out-projection + residual + LN2 + FFN on own tokens -------
            with tc.tile_pool(name="x2pool", bufs=1) as x2p:
                x2own = x2p.tile([128, 8, TOK], F32, tag="x2own")
                x2b = x2p.tile([128, 8, TOK], BF, tag="x2b")

                with (
                    tc.tile_pool(name="ofl", bufs=1) as ofp,
                    tc.tile_pool(name="oflin", bufs=4) as ofi,
                    tc.tile_pool(name="prps", bufs=3, space="PSUM") as prp,
                ):
                    ofull = ofp.tile([128, 8, TOK], BF, tag="ofull")
                    for j in range(4):
                        for pp in range(2):
                            rows = slice(128 * pp, 128 * (pp + 1))
                            olo = ofi.tile([128, TOK], BF, tag="glo")
                            nc.sync.dma_start(olo[:], a2a_out[j, rows, :])
                            ohi = ofi.tile([128, TOK], BF, tag="ghi")
                            nc.sync.dma_start(ohi[:], a2a_out[4 + j, rows, :])
                            nc.vector.tensor_add(
                                ofull[:, 2 * j + pp, :], olo[:], ohi[:])
                    for m in range(8):
                        ps = prp.tile([128, TOK], F32, tag="pr_ps")
                        for kc in range(8):
                            nc.tensor.matmul(
                                ps[:], wp_t[:, kc, m * 128:(m + 1) * 128],
                                ofull[:, kc, :],
                                start=(kc == 0), stop=(kc == 7))
                        nc.vector.scalar_tensor_tensor(
                            x2own[:, m, :], ps[:], bp_t[:, m:m + 1],
                            xo_t[:, m, :], ALU.add, ALU.add)
                        nc.gpsimd.tensor_copy(x2b[:, m, :], x2own[:, m, :])

                with tc.tile_pool(name="ffn", bufs=1) as ffp:
                    h2 = ffp.tile([128, 8, TOK], BF, tag="h2")
                    with (
                        tc.tile_pool(name="l2p", bufs=1) as l2p,
                        tc.tile_pool(name="l2row", bufs=1) as l2row,
                        tc.tile_pool(name="l2ps", bufs=2, space="PSUM") as l2ps,
                    ):
                        layernorm_fm(x2b, TOK, l2p, l2ps, l2row,
                                     lambda kc: h2[:, kc, :])

                    mid = ffp.tile([128, 32, TOK], BF, tag="mid")
                    with (
                        tc.tile_pool(name="w1p", bufs=4) as w1p,
                        tc.tile_pool(name="ffps", bufs=4, space="PSUM") as fps,
                    ):
                        for m in range(32):
                            w1t = w1p.tile([128, 8, 128], BF, tag="w1t")
                            nc.sync.dma_start(w1t[:], w1[m])
                            ps = fps.tile([128, TOK], F32, tag="ff_ps")
                            for kc in range(8):
                                nc.tensor.matmul(
                                    ps[:], w1t[:, kc, :], h2[:, kc, :],
                                    start=(kc == 0), stop=(kc == 7))
                            nc.scalar.activation(mid[:, m, :], ps[:], AF.Relu,
                                                 bias=b1_t[:, m:m + 1])
                    with (
                        tc.tile_pool(name="w2p", bufs=3) as w2p,
                        tc.tile_pool(name="ff2ps", bufs=4, space="PSUM") as fp2,
                        tc.tile_pool(name="yst", bufs=3) as ysp,
                    ):
                        for m in range(8):
                            w2t = w2p.tile([128, 32, 128], BF, tag="w2t")
                            nc.sync.dma_start(w2t[:], w2[m])
                            ps = fp2.tile([128, TOK], F32, tag="ff2_ps")
                            for kc in range(32):
                                nc.tensor.matmul(
                                    ps[:], w2t[:, kc, :], mid[:, kc, :],
                                    start=(kc == 0), stop=(kc == 31))
                            ym = ysp.tile([128, TOK], F32, tag="ym")
                            nc.vector.scalar_tensor_tensor(
                                ym[:], ps[:], b2_t[:, m:m + 1],
                                x2own[:, m, :], ALU.add, ALU.add)
                            nc.sync.dma_start(y[:, m, :], ym[:])

    nc.compile()
    return nc


_NC_CACHE = None


def _get_nc():
    global _NC_CACHE
    if _NC_CACHE is None:
        _NC_CACHE = build_bass()
    return _NC_CACHE


def _fm_tile(a):
    """[C, N] -> [128, C//128, N] (partition-major feature tiling)."""
    Cd, N = a.shape
    return np.ascontiguousarray(a.reshape(Cd // 128, 128, N).transpose(1, 0, 2))


def prepare_inputs(x, Wq, Wk, Wv, Wproj, bproj, ln1_g, ln1_b, ln2_g, ln2_b,
                   W1, b1, W2, b2):
    """Build the 8 per-core input dicts (all numpy, host side)."""
    x = np.asarray(x, np.float32)
    f32 = lambda a: np.asarray(a, np.float32)
    Wq, Wk, Wv = f32(Wq), f32(Wk), f32(Wv)
    Wproj, bproj = f32(Wproj), f32(bproj)
    ln1_g, ln1_b, ln2_g, ln2_b = f32(ln1_g), f32(ln1_b), f32(ln2_g), f32(ln2_b)
    W1, b1, W2, b2 = f32(W1), f32(b1), f32(W2), f32(b2)

    slopes = _alibi_slopes(H)

    # fold LN1 gain/bias into the QKV weights:  h = ln_raw*g + b
    WqF = Wq * ln1_g[None, :, None]      # [H, C, HS]
    WkF = Wk * ln1_g[None, :, None] * (HS ** -0.5)   # fold 1/sqrt(HS) into K
    WvF = Wv * ln1_g[None, :, None]
    bqF = np.einsum("c,hcd->hd", ln1_b, WqF)   # [H, HS]
    bkF = np.einsum("c,hcd->hd", ln1_b, WkF)
    bvF = np.einsum("c,hcd->hd", ln1_b, WvF)
    # fold LN2 gain/bias into W1
    W1F = W1 * ln2_g[:, None]
    b1F = b1 + ln2_b @ W1F

    w1h = np.ascontiguousarray(
        W1F.astype(NP_BF16).reshape(8, 128, 32, 128).transpose(2, 1, 0, 3))
    w2h = np.ascontiguousarray(
        W2.astype(NP_BF16).reshape(32, 128, 8, 128).transpose(2, 1, 0, 3))
    b1h = np.ascontiguousarray(b1F.reshape(32, 128).T)
    b2h = np.ascontiguousarray(b2.reshape(8, 128).T)
    bph = np.ascontiguousarray(bproj.reshape(8, 128).T)
    wph = _fm_tile(Wproj.astype(NP_BF16))      # full [128, 8, 1024]

    in_maps = []
    for c in range(NCORES):
        b = c // 4
        g = c % 4
        mskh = np.zeros((128, 2), np.float32)
        mskh[:, b] = 1.0
        heads = range(4 * g, 4 * g + 4)
        xb = x[b].T                                    # [C, T] feature-major
        wq_own = np.concatenate([WqF[h] for h in heads], axis=1)   # [C, 256]
        wk_own = np.concatenate([WkF[h] for h in heads], axis=1)
        wv_own = np.concatenate([WvF[h] for h in heads], axis=1)
        bq_own = np.concatenate([bqF[h] for h in heads])           # [256]
        bk_own = np.concatenate([bkF[h] for h in heads])
        bv_own = np.concatenate([bvF[h] for h in heads])
        fts = np.stack([_factor_table(slopes[h]) for h in heads])  # [4,128,FW]

        in_maps.append({
            "xfm": _fm_tile(xb),
            "xown": _fm_tile(xb[:, g * TOK:(g + 1) * TOK]),
            "wq": _fm_tile(wq_own.astype(NP_BF16)),
            "wk": _fm_tile(wk_own.astype(NP_BF16)),
            "wv": _fm_tile(wv_own.astype(NP_BF16)),
            "bq": np.ascontiguousarray(bq_own.reshape(2, 128).T.astype(np.float32)),
            "bk": np.ascontiguousarray(bk_own.reshape(2, 128).T.astype(np.float32)),
            "bv": bv_own[None, :].astype(np.float32),
            "wp": wph,
            "bp": bph,
            "ft": fts,
            "w1": w1h,
            "b1": b1h,
            "w2": w2h,
            "b2": b2h,
            "msk": mskh,
        })
    return in_maps


def assemble_output(results):
    out = np.empty((B, T, C), np.float32)
    for c in range(NCORES):
        b, g = c // 4, c % 4
        yc = results[c]["y"]                        # [128, 8, TOK]
        yc = yc.transpose(1, 0, 2).reshape(C, TOK)  # [C, TOK]
        out[b, g * TOK:(g + 1) * TOK, :] = yc.T
    return out


def kernel(**inputs):
    nc = _get_nc()
    in_maps = prepare_inputs(**inputs)
    res = run_bass_kernel_spmd(nc, in_maps, core_ids=list(range(NCORES)))
    return assemble_output(res.results)


if __name__ == "__main__":
    import reference
    ins = {k: np.asarray(v) for k, v in reference.setup_inputs().items()}
    exp = np.asarray(reference.reference(**ins))
    got = kernel(**ins)
    err = np.linalg.norm(got - exp) / np.linalg.norm(exp)
    print("Relative error:", err)


# revision 26
# speedup vs baseline: 1.1428x; 1.0541x over previous
"""Trainium2 Bass kernel for a dense pre-norm transformer block with ALiBi attention.

Reference semantics (B=2, T=2048, C=1024, H=16, HS=64):
    h  = LN1(x);  q,k,v = per-head projections of h
    wei = softmax(causal(q k^T / sqrt(HS) + alibi))
    x  = x + (concat_heads(wei @ v) @ Wproj + bproj)
    x  = x + (relu(LN2(x) @ W1 + b1) @ W2 + b2)

Distribution over 8 NeuronCores: 2-way data parallel over batch (quads
{0..3} and {4..7}) x 4-way tensor parallel over heads within each quad
(4 heads per core).  After attention each core holds its 4 heads' outputs
for all tokens; a small bf16 AllToAll within the quad transposes this to
"all 16 heads for my 512 tokens", after which the attention out-projection
and the FFN run fully local per core (no reduction collective needed).

On-device layout is feature-major ([feature, token]) throughout.  The host
pre-transposes inputs / post-transposes outputs, folds the LN gains/biases
into the adjacent weight matrices, and pre-scales Wk by 1/sqrt(HS).
ALiBi+causal masking is a multiplicative factor table
F[s,t] = exp(-slope*|t-s|) * (s<=t), precomputed on host per head.
The softmax denominator is fused into the AV matmul by appending a ones
column to each head's V block (65-wide stationary operand).
"""

import math

import numpy as np
import ml_dtypes

import concourse.bass as bass
import concourse.mybir as mybir
from concourse import bacc
from concourse.tile import TileContext
from concourse.bass_utils import run_bass_kernel_spmd

B, T, C, H, HS = 2, 2048, 1024, 16, 64
EPS = 1e-5
NCORES = 8
HPC = 4            # heads per core
TOK = 512          # tokens owned per core (FFN/output shard)
FW = 2432          # factor-table width: 384 + 1536 + 512
BF = mybir.dt.bfloat16
F32 = mybir.dt.float32
AF = mybir.ActivationFunctionType
ALU = mybir.AluOpType
NP_BF16 = ml_dtypes.bfloat16


def _alibi_slopes(n_head):
    n = 2 ** int(math.floor(math.log2(n_head)))
    m = np.power(2.0 ** (-8.0 / n), np.arange(1, n + 1))
    if n < n_head:
        m_hat = np.power(2.0 ** (-4.0 / n), np.arange(1, 1 + 2 * (n_head - n), 2))
        m = np.concatenate([m, m_hat])
    return m.astype(np.float64)


def _factor_table(slope):
    """F[i, u]: for tile (s0, t0), F[i, 384+(t0-s0)+j] = alibi*mask at s=s0+i, t=t0+j."""
    i = np.arange(128)[:, None]
    d = np.arange(FW)[None, :] - 384          # d = (t0-s0)+j;  t-s = d-i
    rel = d - i
    f = np.exp(-slope * np.abs(rel))
    f[rel < 0] = 0.0
    return f.astype(NP_BF16)


def build_bass():
    nc = bacc.Bacc("TRN2", debug=False, num_devices=NCORES)

    # ---- I/O ----
    xfm = nc.dram_tensor("xfm", [128, 8, T], F32, kind="ExternalInput")
    xown = nc.dram_tensor("xown", [128, 8, TOK], F32, kind="ExternalInput")
    wq = nc.dram_tensor("wq", [128, 8, 256], BF, kind="ExternalInput")
    wk = nc.dram_tensor("wk", [128, 8, 256], BF, kind="ExternalInput")
    wv = nc.dram_tensor("wv", [128, 8, 256], BF, kind="ExternalInput")
    bq = nc.dram_tensor("bq", [128, 2], F32, kind="ExternalInput")
    bk = nc.dram_tensor("bk", [128, 2], F32, kind="ExternalInput")
    bv = nc.dram_tensor("bv", [1, 256], F32, kind="ExternalInput")
    wp = nc.dram_tensor("wp", [128, 8, 1024], BF, kind="ExternalInput")
    bp = nc.dram_tensor("bp", [128, 8], F32, kind="ExternalInput")
    ft = nc.dram_tensor("ft", [HPC, 128, FW], BF, kind="ExternalInput")
    w1 = nc.dram_tensor("w1", [32, 128, 8, 128], BF, kind="ExternalInput")
    b1 = nc.dram_tensor("b1", [128, 32], F32, kind="ExternalInput")
    w2 = nc.dram_tensor("w2", [8, 128, 32, 128], BF, kind="ExternalInput")
    b2 = nc.dram_tensor("b2", [128, 8], F32, kind="ExternalInput")
    msk = nc.dram_tensor("msk", [128, 2], F32, kind="ExternalInput")
    y = nc.dram_tensor("y", [128, 8, TOK], F32, kind="ExternalOutput")

    with TileContext(nc) as tc:
        with (
            tc.tile_pool(name="const", bufs=1) as cp,
            tc.tile_pool(name="dram", bufs=1, space="DRAM") as dp,
        ):
            ones_bf = cp.tile([128, 1], BF)
            nc.vector.memset(ones_bf[:], 1.0)
            eps_t = cp.tile([1, 1], F32)
            nc.vector.memset(eps_t[:], EPS)
            bq_t = cp.tile([128, 2], F32, tag="bq")
            nc.sync.dma_start(bq_t[:], bq[:])
            bk_t = cp.tile([128, 2], F32, tag="bk")
            nc.sync.dma_start(bk_t[:], bk[:])
            bv_row = cp.tile([1, 256], F32, tag="bvr")
            nc.sync.dma_start(bv_row[:], bv[:])
            bv_b = cp.tile([128, 256], F32, tag="bvb")
            nc.gpsimd.partition_broadcast(bv_b[:], bv_row[:])
            bp_t = cp.tile([128, 8], F32, tag="bp")
            nc.sync.dma_start(bp_t[:], bp[:])
            b1_t = cp.tile([128, 32], F32, tag="b1")
            nc.sync.dma_start(b1_t[:], b1[:])
            b2_t = cp.tile([128, 8], F32, tag="b2")
            nc.sync.dma_start(b2_t[:], b2[:])
            # (loaded later, after the x chunks, to keep the DMA queue clear
            # for the LN1 input at kernel start)
            xo_t = cp.tile([128, 8, TOK], F32, tag="xo")
            wp_t = cp.tile([128, 8, 1024], BF, tag="wp")

            msk_t = cp.tile([128, 2], F32, tag="msk")
            nc.sync.dma_start(msk_t[:], msk[:])

            # The intra-quad head->token transpose runs as an 8-way AllToAll
            # (4-core AllToAll is unsupported).  Each core stages its block
            # masked by its quad indicator into BOTH the low (chunks 0-3,
            # quad-0 destinations) and high (chunks 4-7, quad-1) slots; the
            # wrong-quad copy is zeros, so receivers just add the halves.
            a2a_in = dp.tile([8, 256, TOK], BF)
            a2a_out = dp.tile([8, 256, TOK], BF)

            # -------- per-512-chunk LayerNorm stats -> h = (x-mu)*rstd --------
            # Feature-major: mean/var over the partition(feature) dim via
            # ones-matmuls; rstd via Sqrt + vector reciprocal (no Ln/Exp
            # activation-table thrash); normalize in bf16 split across
            # GpSimd (sub) and Vector (mult).
            def layernorm_fm(xb_sb, W, lp, lps, rowp, write_out):
                xsq = lp.tile([128, 8, W], BF, tag="ln_xsq", bufs=2)
                nc.gpsimd.tensor_tensor(xsq[:], xb_sb[:], xb_sb[:], ALU.mult)
                sx = lps.tile([1, W], F32, tag="ln_sx", bufs=1)
                sq = lps.tile([1, W], F32, tag="ln_sq", bufs=1)
                for kc in range(8):
                    nc.tensor.matmul(sx[:], ones_bf[:], xb_sb[:, kc, :],
                                     start=(kc == 0), stop=(kc == 7))
                for kc in range(8):
                    nc.tensor.matmul(sq[:], ones_bf[:], xsq[:, kc, :],
                                     start=(kc == 0), stop=(kc == 7))
                mu = rowp.tile([1, W], F32, tag="ln_mu", bufs=2)
                nc.scalar.mul(mu[:], sx[:], 1.0 / C)
                musq = rowp.tile([1, W], F32, tag="ln_musq", bufs=2)
                nc.vector.tensor_tensor(musq[:], mu[:], mu[:], ALU.mult)
                sd = rowp.tile([1, W], F32, tag="ln_sd", bufs=2)
                nc.vector.scalar_tensor_tensor(sd[:], sq[:], 1.0 / C, musq[:],
                                               ALU.mult, ALU.subtract)
                nc.scalar.activation(sd[:], sd[:], AF.Sqrt, bias=eps_t[:])
                rstd = rowp.tile([1, W], F32, tag="ln_rstd", bufs=2)
                nc.vector.reciprocal_approx_fast(rstd[:], sd[:])
                mu_bf = rowp.tile([1, W], BF, tag="ln_mubf", bufs=2)
                nc.vector.tensor_copy(mu_bf[:], mu[:])
                rstd_bf = rowp.tile([1, W], BF, tag="ln_rstdbf", bufs=2)
                nc.vector.tensor_copy(rstd_bf[:], rstd[:])
                mub = lp.tile([128, W], BF, tag="ln_mub", bufs=2)
                nc.gpsimd.partition_broadcast(mub[:], mu_bf[:])
                rsb = lp.tile([128, W], BF, tag="ln_rsb", bufs=2)
                nc.gpsimd.partition_broadcast(rsb[:], rstd_bf[:])
                for kc in range(8):
                    tmp = lp.tile([128, W], BF, tag="ln_tmp", bufs=4)
                    if kc % 2 == 0:
                        nc.gpsimd.tensor_sub(tmp[:], xb_sb[:, kc, :], mub[:])
                    else:
                        nc.vector.tensor_sub(tmp[:], xb_sb[:, kc, :], mub[:])
                    nc.vector.tensor_tensor(write_out(kc), tmp[:], rsb[:],
                                            ALU.mult)

            with tc.tile_pool(name="qkvpool", bufs=1) as qp:
                qfm = qp.tile([128, 2, T], BF, tag="qfm")
                kfm = qp.tile([128, 2, T], BF, tag="kfm")
                v_t = qp.tile([128, 16, HPC, 65], BF, tag="v")
                nc.vector.memset(v_t[:, :, :, 64:65], 1.0)

                # ------- LN1 + QKV, pipelined per 512-token chunk -------
                with (
                    tc.tile_pool(name="hpool", bufs=1) as hp,
                    tc.tile_pool(name="xin", bufs=2) as xp,
                    tc.tile_pool(name="lnp", bufs=1) as lp,
                    tc.tile_pool(name="lnrow", bufs=1) as rowp,
                    tc.tile_pool(name="lnps", bufs=2, space="PSUM") as lps,
                    tc.tile_pool(name="wqkv", bufs=1) as wqp,
                    tc.tile_pool(name="qkps", bufs=4, space="PSUM") as qps,
                ):
                    wq_t = wqp.tile([128, 8, 256], BF, tag="wq")
                    nc.sync.dma_start(wq_t[:], wq[:])
                    wk_t = wqp.tile([128, 8, 256], BF, tag="wk")
                    nc.sync.dma_start(wk_t[:], wk[:])
                    wv_t = wqp.tile([128, 8, 256], BF, tag="wv")
                    nc.sync.dma_start(wv_t[:], wv[:])
                    h_t = hp.tile([128, 8, T], BF, tag="h")

                    for ch in range(4):
                        tsl = slice(ch * 512, (ch + 1) * 512)
                        xc = xp.tile([128, 8, 512], F32, tag="xc")
                        nc.sync.dma_start(xc[:], xfm[:, :, tsl])
                        xb = xp.tile([128, 8, 512], BF, tag="xb")
                        nc.vector.tensor_copy(xb[:], xc[:])
                        layernorm_fm(
                            xb, 512, lp, lps, rowp,
                            lambda kc, ch=ch: h_t[:, kc, ch * 512:(ch + 1) * 512])

                        # Q,K for this chunk
                        for p in range(2):
                            ps = qps.tile([128, 512], F32, tag="qk_ps", bufs=3)
                            for kc in range(8):
                                nc.tensor.matmul(
                                    ps[:], wq_t[:, kc, p * 128:(p + 1) * 128],
                                    h_t[:, kc, tsl],
                                    start=(kc == 0), stop=(kc == 7))
                            nc.scalar.add(qfm[:, p, tsl], ps[:], bq_t[:, p:p + 1])
                            ps2 = qps.tile([128, 512], F32, tag="qk_ps",
                                           bufs=3)
                            for kc in range(8):
                                nc.tensor.matmul(
                                    ps2[:], wk_t[:, kc, p * 128:(p + 1) * 128],
                                    h_t[:, kc, tsl],
                                    start=(kc == 0), stop=(kc == 7))
                            nc.scalar.add(kfm[:, p, tsl], ps2[:], bk_t[:, p:p + 1])
                        # V for this chunk (token-major, 128-token blocks)
                        for t4 in range(4):
                            tch = ch * 4 + t4
                            psv = qps.tile([128, 256], F32, tag="v_ps", bufs=2)
                            for kc in range(8):
                                nc.tensor.matmul(
                                    psv[:], h_t[:, kc, tch * 128:(tch + 1) * 128],
                                    wv_t[:, kc, :],
                                    start=(kc == 0), stop=(kc == 7))
                            nc.vector.tensor_add(v_t[:, tch, :, 0:64], psv[:],
                                                 bv_b[:])

                # ---------- Attention ----------  (h freed; qkv + F live)
                with (
                    tc.tile_pool(name="fpool", bufs=1) as fp,
                    tc.tile_pool(name="scps", bufs=2, space="PSUM") as scp,
                    tc.tile_pool(name="oaps", bufs=2, space="PSUM") as oap,
                    tc.tile_pool(name="attp", bufs=3) as atp,
                    tc.tile_pool(name="onrm", bufs=1) as onp,
                ):
                    f_t = []
                    for hh in range(HPC):
                        f = fp.tile([128, FW], BF, tag=f"ft{hh}")
                        nc.sync.dma_start(f[:], ft[hh])
                        f_t.append(f)
                    nc.sync.dma_start(xo_t[:], xown[:])
                    nc.sync.dma_start(wp_t[:], wp[:])

                    for p in range(2):
                        for tcn in range(4):
                            t0 = tcn * 512
                            tsl = slice(t0, t0 + 512)
                            nums = []
                            for hh in range(2):
                                numt = oap.tile([65, 512], F32, tag=f"num{hh}")
                                nums.append(numt)
                            ns = 4 * (tcn + 1)
                            for si in range(ns):
                                s0 = si * 128
                                sc = scp.tile([128, 2, 512], F32, tag="sc")
                                at = atp.tile([128, 2, 512], BF, tag="at")
                                for hh in range(2):
                                    pb = 64 * hh
                                    nc.tensor.matmul(
                                        sc[:, hh, :],
                                        kfm[pb:pb + 64, p, s0:s0 + 128],
                                        qfm[pb:pb + 64, p, tsl],
                                        start=True, stop=True)
                                nc.scalar.activation(at[:], sc[:], AF.Exp)
                                dlt = t0 - s0 + 384
                                am = atp.tile([128, 2, 512], BF, tag="am")
                                nc.vector.tensor_tensor(
                                    am[:, 0, :], at[:, 0, :],
                                    f_t[2 * p][:, dlt:dlt + 512], ALU.mult)
                                nc.gpsimd.tensor_tensor(
                                    am[:, 1, :], at[:, 1, :],
                                    f_t[2 * p + 1][:, dlt:dlt + 512], ALU.mult)
                                st, sp_ = (si == 0), (si == ns - 1)
                                for hh in range(2):
                                    nc.tensor.matmul(
                                        nums[hh][:],
                                        v_t[:, si, 2 * p + hh, :],
                                        am[:, hh, :],
                                        start=st, stop=sp_)
                            # normalize (num/den) and stage for AllToAll
                            for hh in range(2):
                                dsb = onp.tile([65, 512], F32, tag="dsb", bufs=3)
                                nc.vector.tensor_copy(dsb[64:65, :],
                                                      nums[hh][64:65, :])
                                den = onp.tile([1, 512], F32, tag="den", bufs=3)
                                nc.sync.dma_start(den[:], dsb[64:65, :])
                                rec = onp.tile([1, 512], F32, tag="rec", bufs=3)
                                nc.vector.reciprocal_approx_fast(rec[:], den[:])
                                rb = onp.tile([64, 512], F32, tag="rb", bufs=3)
                                nc.gpsimd.partition_broadcast(rb[:], rec[:])
                                ofh = onp.tile([64, 512], BF, tag="ofh", bufs=3)
                                nc.vector.tensor_tensor(
                                    ofh[:], nums[hh][0:64, :], rb[:], ALU.mult)
                                rows = slice(p * 128 + 64 * hh,
                                             p * 128 + 64 * hh + 64)
                                nc.sync.dma_start(a2a_in[tcn, rows, :], ofh[:])
                                nc.sync.dma_start(a2a_in[4 + tcn, rows, :],
                                                  ofh[:])

            # ---- tiny bf16 AllToAll within each quad: heads -> tokens ----
            nc.gpsimd.collective_compute(
                "AllToAll", ALU.bypass,
                replica_groups=[[0, 1, 2, 3, 4, 5, 6, 7]],
                ins=[a2a_in.opt()], outs=[a2a_out.opt()])

            # ------- out-projection + residual + LN2 + FFN on own tokens -------
            with tc.tile_pool(name="x2pool", bufs=1) as x2p:
                x2own = x2p.tile([128, 8, TOK], F32, tag="x2own")
                x2b = x2p.tile([128, 8, TOK], BF, tag="x2b")

                with (
                    tc.tile_pool(name="ofl", bufs=1) as ofp,
                    tc.tile_pool(name="oflin", bufs=4) as ofi,
                    tc.tile_pool(name="prps", bufs=3, space="PSUM") as prp,
                ):
                    ofull = ofp.tile([128, 8, TOK], BF, tag="ofull")
                    # both halves arrive unmasked (quad-0 sources in chunks
                    # 0-3, quad-1 in 4-7); select the own-quad half via the
                    # per-core 0/1 mask columns.
                    for j in range(4):
                        for pp in range(2):
                            rows = slice(128 * pp, 128 * (pp + 1))
                            olo = ofi.tile([128, TOK], BF, tag="glo")
                            nc.sync.dma_start(olo[:], a2a_out[j, rows, :])
                            ohi = ofi.tile([128, TOK], BF, tag="ghi")
                            nc.sync.dma_start(ohi[:], a2a_out[4 + j, rows, :])
                            hsel = ofi.tile([128, TOK], BF, tag="hsel")
                            nc.scalar.mul(hsel[:], ohi[:], msk_t[:, 1:2])
                            nc.vector.scalar_tensor_tensor(
                                ofull[:, 2 * j + pp, :], olo[:],
                                msk_t[:, 0:1], hsel[:], ALU.mult, ALU.add)
                    for m in range(8):
                        ps = prp.tile([128, TOK], F32, tag="pr_ps")
                        for kc in range(8):
                            nc.tensor.matmul(
                                ps[:], wp_t[:, kc, m * 128:(m + 1) * 128],
                                ofull[:, kc, :],
                                start=(kc == 0), stop=(kc == 7))
                        nc.vector.scalar_tensor_tensor(
                            x2own[:, m, :], ps[:], bp_t[:, m:m + 1],
                            xo_t[:, m, :], ALU.add, ALU.add)
                        nc.scalar.copy(x2b[:, m, :], x2own[:, m, :])

                with tc.tile_pool(name="ffn", bufs=1) as ffp:
                    h2 = ffp.tile([128, 8, TOK], BF, tag="h2")
                    with (
                        tc.tile_pool(name="l2p", bufs=1) as l2p,
                        tc.tile_pool(name="l2row", bufs=1) as l2row,
                        tc.tile_pool(name="l2ps", bufs=2, space="PSUM") as l2ps,
                    ):
                        layernorm_fm(x2b, TOK, l2p, l2ps, l2row,
                                     lambda kc: h2[:, kc, :])

                    mid = ffp.tile([128, 32, TOK], BF, tag="mid")
                    with (
                        tc.tile_pool(name="w1p", bufs=4) as w1p,
                        tc.tile_pool(name="ffps", bufs=4, space="PSUM") as fps,
                    ):
                        for m in range(32):
                            w1t = w1p.tile([128, 8, 128], BF, tag="w1t")
                            nc.sync.dma_start(w1t[:], w1[m])
                            ps = fps.tile([128, TOK], F32, tag="ff_ps")
                            for kc in range(8):
                                nc.tensor.matmul(
                                    ps[:], w1t[:, kc, :], h2[:, kc, :],
                                    start=(kc == 0), stop=(kc == 7))
                            nc.scalar.activation(mid[:, m, :], ps[:], AF.Relu,
                                                 bias=b1_t[:, m:m + 1])
                    with (
                        tc.tile_pool(name="w2p", bufs=3) as w2p,
                        tc.tile_pool(name="ff2ps", bufs=4, space="PSUM") as fp2,
                        tc.tile_pool(name="yst", bufs=3) as ysp,
                    ):
                        for m in range(8):
                            w2t = w2p.tile([128, 32, 128], BF, tag="w2t")
                            nc.sync.dma_start(w2t[:], w2[m])
                            ps = fp2.tile([128, TOK], F32, tag="ff2_ps")
                            for kc in range(32):
                                nc.tensor.matmul(
                                    ps[:], w2t[:, kc, :], mid[:, kc, :],
                                    start=(kc == 0), stop=(kc == 31))
                            ym = ysp.tile([128, TOK], F32, tag="ym")
                            nc.vector.scalar_tensor_tensor(
                                ym[:], ps[:], b2_t[:, m:m + 1],
                                x2own[:, m, :], ALU.add, ALU.add)
                            nc.sync.dma_start(y[:, m, :], ym[:])

    nc.compile()
    return nc


_NC_CACHE = None


def _get_nc():
    global _NC_CACHE
    if _NC_CACHE is None:
        _NC_CACHE = build_bass()
    return _NC_CACHE


def _fm_tile(a):
    """[C, N] -> [128, C//128, N] (partition-major feature tiling)."""
    Cd, N = a.shape
    return np.ascontiguousarray(a.reshape(Cd // 128, 128, N).transpose(1, 0, 2))


def prepare_inputs(x, Wq, Wk, Wv, Wproj, bproj, ln1_g, ln1_b, ln2_g, ln2_b,
                   W1, b1, W2, b2):
    """Build the 8 per-core input dicts (all numpy, host side)."""
    x = np.asarray(x, np.float32)
    f32 = lambda a: np.asarray(a, np.float32)
    Wq, Wk, Wv = f32(Wq), f32(Wk), f32(Wv)
    Wproj, bproj = f32(Wproj), f32(bproj)
    ln1_g, ln1_b, ln2_g, ln2_b = f32(ln1_g), f32(ln1_b), f32(ln2_g), f32(ln2_b)
    W1, b1, W2, b2 = f32(W1), f32(b1), f32(W2), f32(b2)

    slopes = _alibi_slopes(H)

    # fold LN1 gain/bias into the QKV weights:  h = ln_raw*g + b
    WqF = Wq * ln1_g[None, :, None]      # [H, C, HS]
    WkF = Wk * ln1_g[None, :, None] * (HS ** -0.5)   # fold 1/sqrt(HS) into K
    WvF = Wv * ln1_g[None, :, None]
    bqF = np.einsum("c,hcd->hd", ln1_b, WqF)   # [H, HS]
    bkF = np.einsum("c,hcd->hd", ln1_b, WkF)
    bvF = np.einsum("c,hcd->hd", ln1_b, WvF)
    # fold LN2 gain/bias into W1
    W1F = W1 * ln2_g[:, None]
    b1F = b1 + ln2_b @ W1F

    w1h = np.ascontiguousarray(
        W1F.astype(NP_BF16).reshape(8, 128, 32, 128).transpose(2, 1, 0, 3))
    w2h = np.ascontiguousarray(
        W2.astype(NP_BF16).reshape(32, 128, 8, 128).transpose(2, 1, 0, 3))
    b1h = np.ascontiguousarray(b1F.reshape(32, 128).T)
    b2h = np.ascontiguousarray(b2.reshape(8, 128).T)
    bph = np.ascontiguousarray(bproj.reshape(8, 128).T)
    wph = _fm_tile(Wproj.astype(NP_BF16))      # full [128, 8, 1024]

    in_maps = []
    for c in range(NCORES):
        b = c // 4
        g = c % 4
        mskh = np.zeros((128, 2), np.float32)
        mskh[:, b] = 1.0
        heads = range(4 * g, 4 * g + 4)
        xb = x[b].T                                    # [C, T] feature-major
        wq_own = np.concatenate([WqF[h] for h in heads], axis=1)   # [C, 256]
        wk_own = np.concatenate([WkF[h] for h in heads], axis=1)
        wv_own = np.concatenate([WvF[h] for h in heads], axis=1)
        bq_own = np.concatenate([bqF[h] for h in heads])           # [256]
        bk_own = np.concatenate([bkF[h] for h in heads])
        bv_own = np.concatenate([bvF[h] for h in heads])
        fts = np.stack([_factor_table(slopes[h]) for h in heads])  # [4,128,FW]

        in_maps.append({
            "xfm": _fm_tile(xb),
            "xown": _fm_tile(xb[:, g * TOK:(g + 1) * TOK]),
            "wq": _fm_tile(wq_own.astype(NP_BF16)),
            "wk": _fm_tile(wk_own.astype(NP_BF16)),
            "wv": _fm_tile(wv_own.astype(NP_BF16)),
            "bq": np.ascontiguousarray(bq_own.reshape(2, 128).T.astype(np.float32)),
            "bk": np.ascontiguousarray(bk_own.reshape(2, 128).T.astype(np.float32)),
            "bv": bv_own[None, :].astype(np.float32),
            "wp": wph,
            "bp": bph,
            "ft": fts,
            "w1": w1h,
            "b1": b1h,
            "w2": w2h,
            "b2": b2h,
            "msk": mskh,
        })
    return in_maps


def assemble_output(results):
    out = np.empty((B, T, C), np.float32)
    for c in range(NCORES):
        b, g = c // 4, c % 4
        yc = results[c]["y"]                        # [128, 8, TOK]
        yc = yc.transpose(1, 0, 2).reshape(C, TOK)  # [C, TOK]
        out[b, g * TOK:(g + 1) * TOK, :] = yc.T
    return out


def kernel(**inputs):
    nc = _get_nc()
    in_maps = prepare_inputs(**inputs)
    res = run_bass_kernel_spmd(nc, in_maps, core_ids=list(range(NCORES)))
    return assemble_output(res.results)


if __name__ == "__main__":
    import reference
    ins = {k: np.asarray(v) for k, v in reference.setup_inputs().items()}
    exp = np.asarray(reference.reference(**ins))
    got = kernel(**ins)
    err = np.linalg.norm(got - exp) / np.linalg.norm(exp)
    print("Relative error:", err)
